# revision 1
# baseline (speedup 1.0000x reference)
"""Multi-head attention (B=2, S=2048, E=1024, H=16, D=64) on 8 trn2 cores.

Sharding: tensor-parallel over heads (2 heads/core). Each core:
  - projects Q/K (fp32) and V (bf16) for its 2 heads over all 4096 tokens
  - computes scores row-packed (2 heads in the PE array), fp32
  - stable softmax: DVE copy+rowmax (fused), ACT exp (fused bias/accum), DVE norm
  - transposes probs via DMA-xbar (SBUF->SBUF, bf16)
  - attn@V col-packed (2 heads), bf16 -> mh^T [128 dims, 4096 tokens]
  - AllToAll to re-shard from head-split to token-split
  - out_proj (bf16) + bias for its 512 tokens -> y chunk [512, 1024] fp32
Host: pre-transposes x, stacks per-core weights, gathers y chunks.
"""

import math
import numpy as np
import ml_dtypes

N_CORES = 8
B, S, E, H, D = 2, 2048, 1024, 16, 64
HPC = H // N_CORES            # heads per core = 2
M = HPC * D                   # stacked head dim = 128
TOK = B * S                   # 4096
TPC = TOK // N_CORES          # tokens per core for out_proj = 512
KT = E // 128                 # contraction tiles for E = 8

_CACHE = {}


def _build_program():
    import os
    import concourse.bacc as bacc
    import concourse.tile as tile
    import concourse.mybir as mybir

    OPT = set(os.environ.get("K_OPT", "").split(","))
    SKIP = set(os.environ.get("K_SKIP", "").split(","))
    # default-safe choices (validated on hw): no accum_out on exp, no
    # tile_position row packing for fp32 scores, no fused TT-reduce
    if "accum" not in OPT:
        SKIP.add("noaccum")
    if "rowpack" not in OPT:
        SKIP.add("rowpack")
    if "ttr" not in OPT:
        SKIP.add("ttr")

    F32 = mybir.dt.float32
    BF16 = mybir.dt.bfloat16
    AF = mybir.ActivationFunctionType
    ALU = mybir.AluOpType
    AX = mybir.AxisListType

    nc = bacc.Bacc("TRN2", target_bir_lowering=False, debug=False,
                   num_devices=N_CORES)

    # ---- I/O ----
    xT_d = nc.dram_tensor("xT", [E, TOK], F32, kind="ExternalInput")
    xTb_d = nc.dram_tensor("xTb", [E, TOK], BF16, kind="ExternalInput")
    wq_d = nc.dram_tensor("wq", [E, M], F32, kind="ExternalInput")
    wk_d = nc.dram_tensor("wk", [E, M], F32, kind="ExternalInput")
    wv_d = nc.dram_tensor("wv", [E, M], BF16, kind="ExternalInput")
    woT_d = nc.dram_tensor("woT", [E, E], BF16, kind="ExternalInput")
    bo_d = nc.dram_tensor("bo", [1, E], BF16, kind="ExternalInput")
    y_d = nc.dram_tensor("y", [TPC, E], F32, kind="ExternalOutput")
    # tiny passthrough used by the benchmark harness to chain executions
    tin_d = nc.dram_tensor("t_in", [1, 4], F32, kind="ExternalInput")
    tout_d = nc.dram_tensor("t_out", [1, 4], F32, kind="ExternalOutput")

    with tile.TileContext(nc) as tc:
        with (
            tc.tile_pool(name="wpool", bufs=1) as wpool,
            tc.tile_pool(name="qkv_sb", bufs=1) as qkv_sb,
            tc.tile_pool(name="xc_pool", bufs=2) as xc_pool,
            tc.tile_pool(name="sm_pool", bufs=2) as sm_pool,
            tc.tile_pool(name="small", bufs=4) as small,
            tc.tile_pool(name="ptp", bufs=3) as ptp,
            tc.tile_pool(name="mh_pool", bufs=2) as mh_pool,
            tc.tile_pool(name="scores_ps", bufs=3, space="PSUM") as scores_ps,
            tc.tile_pool(name="mm512", bufs=2, space="PSUM") as mm512,
            tc.tile_pool(name="dram", bufs=1, space="DRAM") as dram,
        ):
            # ---- weights to SBUF ----
            wq_sb = wpool.tile([128, KT, M], F32, name="wq_sb")
            wk_sb = wpool.tile([128, KT, M], F32, name="wk_sb")
            wv_sb = wpool.tile([128, KT, M], BF16, name="wv_sb")
            woT_sb = wpool.tile([128, KT, E], BF16, name="woT_sb")
            bo_sb = wpool.tile([1, E], BF16, name="bo_sb")
            ones_sb = wpool.tile([1, 128], BF16, name="ones_sb")
            for k in range(KT):
                nc.sync.dma_start(wq_sb[:, k, :], wq_d[k * 128:(k + 1) * 128, :])
                nc.sync.dma_start(wk_sb[:, k, :], wk_d[k * 128:(k + 1) * 128, :])
                nc.sync.dma_start(wv_sb[:, k, :], wv_d[k * 128:(k + 1) * 128, :])
                nc.sync.dma_start(woT_sb[:, k, :], woT_d[k * 128:(k + 1) * 128, :])
            nc.sync.dma_start(bo_sb[:], bo_d[:])
            nc.vector.memset(ones_sb[:], 1.0)
            t_sb = wpool.tile([1, 4], F32, name="t_sb")
            nc.sync.dma_start(t_sb[:], tin_d[:])
            nc.sync.dma_start(tout_d[:], t_sb[:])
            zeros_sb = wpool.tile([128, 1024], F32, name="zeros_sb")
            nc.vector.memset(zeros_sb[:], 0.0)

            # persistent per-batch activations
            SDT = F32 if "fp32s" in OPT else mybir.dt.float32r
            QT = [qkv_sb.tile([128, S], SDT, name=f"QT{b}") for b in range(B)]
            KTt = [qkv_sb.tile([128, S], SDT, name=f"KTt{b}") for b in range(B)]
            V = [qkv_sb.tile([128, 16, M], BF16, name=f"V{b}") for b in range(B)]

            a2a_in = dram.tile([N_CORES, 128, TPC], BF16, name="a2a_in")
            a2a_out = dram.tile([N_CORES, 128, TPC], BF16, name="a2a_out")

            def qkv_phase(b):
                for tcn in range(4):          # 512-token chunks within batch
                    t0 = b * S + tcn * 512
                    xk = xc_pool.tile([128, KT, 512], F32, name="xk", tag="xk")
                    xbk = xc_pool.tile([128, KT, 512], BF16, name="xbk", tag="xbk")
                    for k in range(KT):
                        nc.sync.dma_start(xk[:, k, :],
                                          xT_d[k * 128:(k + 1) * 128, t0:t0 + 512])
                        nc.sync.dma_start(xbk[:, k, :],
                                          xTb_d[k * 128:(k + 1) * 128, t0:t0 + 512])
                    # Q^T, K^T : [dims 128, tokens 512]
                    for dst, w in ((QT[b], wq_sb), (KTt[b], wk_sb)):
                        ps = mm512.tile([128, 512], F32, name="qk_ps", tag="mm")
                        for k in range(KT):
                            nc.tensor.matmul(ps[:], w[:, k, :], xk[:, k, :],
                                             start=(k == 0), stop=(k == KT - 1))
                        nc.scalar.copy(dst[:, tcn * 512:(tcn + 1) * 512], ps[:])
                    # V: [tokens 128, dims 128] per token-block
                    for tb in range(4):
                        g = tcn * 4 + tb      # block index within batch (0..15)
                        ps = mm512.tile([128, 128], F32, name="v_ps", tag="mm")
                        for k in range(KT):
                            nc.tensor.matmul(
                                ps[:], xbk[:, k, tb * 128:(tb + 1) * 128],
                                wv_sb[:, k, :],
                                start=(k == 0), stop=(k == KT - 1))
                        nc.scalar.copy(V[b][:, g, :], ps[:])

            def att_phase(b):
                F32R = mybir.dt.float32r
                for qg in range(4):           # 512-query groups
                    pT = [ptp.tile([128, 16, 512], BF16, name="pT", tag="pT")
                          for _ in range(HPC)]
                    for qb4 in range(4):
                        qb = qg * 4 + qb4     # query block (0..15)
                        for h in range(HPC):
                            mx2 = small.tile([128, 2], F32, name="mx2",
                                             tag="mx2")
                            sp = []
                            for half in range(2):
                                s = scores_ps.tile([128, 1024], F32, name="sp",
                                                   tag="sp")
                                sp.append(s)
                                qap = QT[b][h * 64:(h + 1) * 64,
                                            qb * 128:(qb + 1) * 128]
                                for c in range(2):
                                    kap = KTt[b][h * 64:(h + 1) * 64,
                                                 half * 1024 + c * 512:
                                                 half * 1024 + (c + 1) * 512]
                                    nc.tensor.matmul(
                                        s[:, c * 512:(c + 1) * 512], qap, kap,
                                        start=True, stop=True)
                                nc.vector.tensor_reduce(
                                    mx2[:, half:half + 1], s[:],
                                    axis=AX.X, op=ALU.max)
                            negm = small.tile([128, 1], F32, name="negm",
                                              tag="negm")
                            nc.vector.tensor_reduce(
                                negm[:], mx2[:], axis=AX.X, op=ALU.max,
                                negate=True)
                            probs = sm_pool.tile([128, S], BF16, name="probs",
                                                 tag="probs")
                            for half in range(2):
                                nc.scalar.activation(
                                    probs[:, half * 1024:(half + 1) * 1024],
                                    sp[half][:], AF.Exp,
                                    bias=negm[:], scale=1.0)
                            sumexp = small.tile([128, 1], F32, name="sumexp",
                                                tag="sumexp")
                            nc.vector.tensor_reduce(
                                sumexp[:], probs[:], axis=AX.X, op=ALU.add)
                            r = small.tile([128, 1], F32, name="r", tag="r")
                            nc.vector.reciprocal(r[:], sumexp[:])
                            if "gnorm" in OPT:
                                nc.gpsimd.tensor_scalar_mul(probs[:], probs[:],
                                                            r[:])
                            else:
                                nc.vector.tensor_scalar_mul(probs[:], probs[:],
                                                            r[:])
                            nc.sync.dma_start(
                                pT[h][:, :, qb4 * 128:(qb4 + 1) * 128],
                                probs[:], transpose=True)
                    # attn @ V for this query group, col-packed heads
                    mh_ps = mm512.tile([128, 512], F32, name="mh_ps", tag="mm")
                    for j in range(16):
                        for h in range(HPC):
                            nc.tensor.matmul(
                                mh_ps[h * 64:(h + 1) * 64, :],
                                V[b][:, j, h * 64:(h + 1) * 64],
                                pT[h][:, j, :],
                                start=(j == 0), stop=(j == 15),
                                tile_position=(0, 64 * h))
                    mh_sb = mh_pool.tile([128, 512], BF16, name="mh_sb", tag="mh")
                    nc.scalar.copy(mh_sb[:], mh_ps[:])
                    nc.sync.dma_start(a2a_in[b * 4 + qg], mh_sb[:])

            import os

            def outproj():
                op_sb = wpool.tile([128, N_CORES, TPC], BF16, name="op_sb")
                for j in range(N_CORES):
                    nc.sync.dma_start(op_sb[:, j, :], a2a_out[j])
                for tb in range(4):
                    y_sb = mh_pool.tile([128, E], F32, name="y_sb", tag="y_sb")
                    for half in range(2):
                        y_ps = mm512.tile([128, 512], F32, name="y_ps", tag="mm")
                        for j in range(N_CORES):
                            nc.tensor.matmul(
                                y_ps[:], op_sb[:, j, tb * 128:(tb + 1) * 128],
                                woT_sb[:, j, half * 512:(half + 1) * 512],
                                start=(j == 0),
                                stop=(j == N_CORES - 1 and "bias" in SKIP))
                        if "bias" not in SKIP:
                            nc.tensor.matmul(
                                y_ps[:], ones_sb[:],
                                bo_sb[:, half * 512:(half + 1) * 512],
                                start=False, stop=True)
                        nc.scalar.copy(y_sb[:, half * 512:(half + 1) * 512],
                                       y_ps[:])
                    nc.sync.dma_start(y_d[tb * 128:(tb + 1) * 128, :], y_sb[:])

            def body():
                for b in range(B):
                    qkv_phase(b)
                    if "att" not in SKIP:
                        att_phase(b)

            loopn = int(os.environ.get("K_LOOP", "0"))
            if loopn:
                with tc.For_i(0, loopn, 1):
                    body()
                    nc.sync.dma_start(a2a_out[:], a2a_in[:])
                    outproj()
            else:
                body()
                if os.environ.get("K_NO_A2A"):
                    nc.sync.dma_start(a2a_out[:], a2a_in[:])
                else:
                    nc.gpsimd.collective_compute(
                        "AllToAll", mybir.AluOpType.bypass,
                        replica_groups=[list(range(N_CORES))],
                        ins=[a2a_in.opt()],
                        outs=[a2a_out.opt()],
                    )
                outproj()

    nc.compile()
    return nc


def _prepare_in_maps(x, W_q, W_k, W_v, W_o, b_o):
    bf = ml_dtypes.bfloat16
    x2 = np.ascontiguousarray(x.reshape(TOK, E).T)          # [E, TOK] f32
    x2b = x2.astype(bf)
    woT = np.ascontiguousarray(W_o.T).astype(bf)            # [E_in, E_out]
    bo = b_o.reshape(1, E).astype(bf)
    scale = np.float32(1.0 / math.sqrt(D))
    in_maps = []
    for c in range(N_CORES):
        h0 = c * HPC
        wq = np.concatenate([W_q[h0 + i] for i in range(HPC)], axis=1) * scale
        wk = np.concatenate([W_k[h0 + i] for i in range(HPC)], axis=1)
        wv = np.concatenate([W_v[h0 + i] for i in range(HPC)], axis=1).astype(bf)
        in_maps.append({
            "xT": x2, "xTb": x2b,
            "wq": np.ascontiguousarray(wq, np.float32),
            "wk": np.ascontiguousarray(wk, np.float32),
            "wv": np.ascontiguousarray(wv),
            "woT": woT, "bo": bo,
            "t_in": np.zeros((1, 4), np.float32),
        })
    return in_maps


def kernel(x, W_q, W_k, W_v, W_o, b_o):
    from concourse.bass_utils import run_bass_kernel_spmd

    if "nc" not in _CACHE:
        _CACHE["nc"] = _build_program()
    nc = _CACHE["nc"]
    in_maps = _prepare_in_maps(x, W_q, W_k, W_v, W_o, b_o)
    res = run_bass_kernel_spmd(nc, in_maps, core_ids=list(range(N_CORES)))
    y = np.concatenate([np.asarray(res.results[c]["y"]) for c in range(N_CORES)],
                       axis=0)
    return np.ascontiguousarray(y.reshape(B, S, E).astype(np.float32))

